# revision 22
# baseline (speedup 1.0000x reference)
"""GNN (MLP + 2x GCNConv + head) on 8 Trainium2 NeuronCores.

Sharding: nodes split 8 ways (12544 per core, padded from 100000 to 100352).
Per conv: transform on PE (feature-major), scale cols by dinv, PE-transpose
to node-major, convert to bf16, AllGather the bf16 table, then per 128-dst
tile: indirect-DMA gather of 128 source rows per chunk (bf16, 256B rows),
one-hot built ON-CHIP by DVE (is_equal vs iota) from a packed dst-offset
stream, scatter-add via bf16 matmul into a [128dst,128feat] PSUM tile,
evacuation adds self-loop from the SBUF-resident local shard + bias + relu,
then inline PE-transpose feeds the next stage (no DRAM round trip).
All edge bookkeeping (dst-sorted chunked index/offset streams) precomputed
on host at first call and cached.
"""
import numpy as np

N_NODES = 100000
N_PAD = 100352          # 8 * 12544
SH = 12544              # nodes per core (98 tiles of 128)
NT = 98                 # 128-node tiles per core
CHUNK = 128             # edges per matmul chunk
HID = 128
NCORES = 8

_cache = {}
TRACE = False      # set True (e.g. from test.py) to capture an NTFF profile
LAST = None        # BassKernelResults of the most recent run


def _prep(edge_index):
    src = np.asarray(edge_index[0], dtype=np.int64)
    dst = np.asarray(edge_index[1], dtype=np.int64)
    deg = np.bincount(dst, minlength=N_PAD).astype(np.float64) + 1.0
    dinv = (1.0 / np.sqrt(deg)).astype(np.float32)  # pad nodes -> 1.0

    core_of = dst // SH
    ch_t = np.zeros((NCORES, NT), dtype=np.int64)
    edata = []
    for c in range(NCORES):
        m = core_of == c
        s = src[m]
        dl = dst[m] - c * SH
        o = np.argsort(dl, kind="stable")
        s, dl = s[o], dl[o]
        t = dl // 128
        cnt = np.bincount(t, minlength=NT)
        ch_t[c] = (cnt + CHUNK - 1) // CHUNK
        edata.append((s, dl, cnt))
    CH = np.maximum(ch_t.max(axis=0), 1)        # chunks per tile (shared)
    TOTCH = int(CH.sum())
    chunk_off = np.concatenate([[0], np.cumsum(CH)])

    import ml_dtypes
    idxs = np.zeros((NCORES, 128, TOTCH), dtype=np.int32)
    dstoff = np.full((NCORES, 128, TOTCH), 128.0, dtype=ml_dtypes.bfloat16)
    for c in range(NCORES):
        s, dl, cnt = edata[c]
        tstart = np.concatenate([[0], np.cumsum(cnt)])
        pos_in_t = np.arange(len(dl)) - tstart[dl // 128]
        ch_local = pos_in_t // CHUNK
        lane = pos_in_t % CHUNK
        gch = chunk_off[dl // 128] + ch_local
        idxs[c, lane, gch] = s.astype(np.int32)
        dstoff[c, lane, gch] = (dl % 128).astype(np.float32)
    return dinv, TOTCH, CH, chunk_off, idxs, dstoff


def _build(TOTCH, CH, chunk_off):
    import concourse.bacc as bacc
    import concourse.bass as bass
    import concourse.mybir as mybir
    import concourse.tile as tile
    from concourse.masks import make_identity

    f32 = mybir.dt.float32
    bf16 = mybir.dt.bfloat16
    i32 = mybir.dt.int32
    RELU = mybir.ActivationFunctionType.Relu
    COPY = mybir.ActivationFunctionType.Copy
    EQ = mybir.AluOpType.is_equal

    nc = bacc.Bacc("TRN2", target_bir_lowering=False, debug=False,
                   enable_asserts=False, num_devices=NCORES)

    xT = nc.dram_tensor("xT", [5, SH], bf16, kind="ExternalInput")
    idxs = nc.dram_tensor("idxs", [128, TOTCH], i32, kind="ExternalInput")
    dstoff = nc.dram_tensor("dstoff", [128, TOTCH], bf16, kind="ExternalInput")
    dinv_cols = nc.dram_tensor("dinv_cols", [128, NT], f32, kind="ExternalInput")
    iota_t = nc.dram_tensor("iota_t", [128, 128], bf16, kind="ExternalInput")
    wts = {}
    for nm, shp, dt in [
            ("w1T", [5, 64], bf16), ("w2T", [64, 128], bf16),
            ("w3T", [128, 128], bf16), ("w4T", [128, 128], bf16),
            ("wc1T", [128, 128], bf16), ("wc2T", [128, 128], bf16),
            ("w5T", [128, 60], bf16),
            ("b1c", [64, 1], f32), ("b2c", [128, 1], f32),
            ("b3c", [128, 1], f32), ("b4c", [128, 1], f32),
            ("b5c", [60, 1], f32),
            ("bc1b", [128, 128], f32), ("bc2b", [128, 128], f32)]:
        wts[nm] = nc.dram_tensor(nm, shp, dt, kind="ExternalInput")
    out = nc.dram_tensor("out", [SH, 60], f32, kind="ExternalOutput")

    with tile.TileContext(nc) as tc:
        with tc.tile_pool(name="w", bufs=1) as wp, \
             tc.tile_pool(name="act", bufs=2) as actp, \
             tc.tile_pool(name="hb", bufs=2) as hbp, \
             tc.tile_pool(name="sh", bufs=1) as shp_, \
             tc.tile_pool(name="xs", bufs=2) as xsp, \
             tc.tile_pool(name="sm", bufs=6) as smp, \
             tc.tile_pool(name="oh", bufs=8) as ohp, \
             tc.tile_pool(name="gat", bufs=64) as gatp, \
             tc.tile_pool(name="mm", bufs=2, space="PSUM") as mmp, \
             tc.tile_pool(name="tr", bufs=2, space="PSUM") as trp, \
             tc.tile_pool(name="agg", bufs=4, space="PSUM") as aggp, \
             tc.tile_pool(name="dram", bufs=1, space="DRAM") as dramp:

            W = {}
            for nm in wts:
                W[nm] = wp.tile(list(wts[nm].shape), wts[nm].dtype,
                                tag=nm, name=nm + "_sb")
                nc.sync.dma_start(out=W[nm][:], in_=wts[nm][:])
            dinv_sb = wp.tile([128, NT], f32, tag="dinv", name="dinv_sb")
            nc.sync.dma_start(out=dinv_sb[:], in_=dinv_cols[:])
            iota_sb = wp.tile([128, 128], bf16, tag="iota", name="iota_sb")
            nc.sync.dma_start(out=iota_sb[:], in_=iota_t[:])
            ident = wp.tile([128, 128], f32, tag="ident", name="ident")
            make_identity(nc, ident[:])
            ident_bf = wp.tile([128, 128], bf16, tag="identb", name="ident_bf")
            nc.scalar.activation(ident_bf[:], ident[:], COPY)
            idx_sb = wp.tile([128, TOTCH], i32, tag="idx", name="idx_sb")
            nc.sync.dma_start(out=idx_sb[:], in_=idxs[:])
            doff_sb = wp.tile([128, TOTCH], bf16, tag="doff", name="doff_sb")
            nc.sync.dma_start(out=doff_sb[:], in_=dstoff[:])

            ag_in = dramp.tile([SH, HID], bf16, name="ag_in")
            ag_out = dramp.tile([N_PAD, HID], bf16, name="ag_out",
                                addr_space="Shared")
            ag_in2 = dramp.tile([SH, HID], bf16, name="ag_in2")
            ag_out2 = dramp.tile([N_PAD, HID], bf16, name="ag_out2",
                                 addr_space="Shared")

            slices = [(s, min(512, SH - s)) for s in range(0, SH, 512)]

            def mlp_layer(dst_t, w_t, b_t, src_t, kin, kout, resid=None):
                for s0, sw in slices:
                    ps = mmp.tile([128, 512], f32, space="PSUM", tag="mm")
                    nc.tensor.matmul(ps[:kout, :sw], lhsT=w_t[:],
                                     rhs=src_t[:kin, s0:s0 + sw],
                                     start=True, stop=True)
                    nc.scalar.activation(dst_t[:kout, s0:s0 + sw],
                                         ps[:kout, :sw], RELU, bias=b_t[:])
                    if resid is not None:
                        nc.vector.tensor_add(dst_t[:kout, s0:s0 + sw],
                                             dst_t[:kout, s0:s0 + sw],
                                             resid[:kout, s0:s0 + sw])

            # ---- MLP (feature-major, fp32) ----
            hA = actp.tile([128, SH], bf16, tag="act", name="hA")
            for s0, sw in slices:
                xt = xsp.tile([5, 512], bf16, tag="xs", name="xt")
                nc.sync.dma_start(out=xt[:, :sw], in_=xT[:, s0:s0 + sw])
                ps = mmp.tile([128, 512], f32, space="PSUM", tag="mm")
                nc.tensor.matmul(ps[:64, :sw], lhsT=W["w1T"][:], rhs=xt[:5, :sw],
                                 start=True, stop=True)
                nc.scalar.activation(hA[:64, s0:s0 + sw], ps[:64, :sw], RELU,
                                     bias=W["b1c"][:])
            hB = actp.tile([128, SH], bf16, tag="act", name="hB")
            mlp_layer(hB, W["w2T"], W["b2c"], hA, 64, 128)
            hC = actp.tile([128, SH], bf16, tag="act", name="hC")
            mlp_layer(hC, W["w3T"], W["b3c"], hB, 128, 128, resid=hB)
            hD = actp.tile([128, SH], bf16, tag="act", name="hD")
            mlp_layer(hD, W["w4T"], W["b4c"], hC, 128, 128, resid=hC)

            def conv(h_fm, wc_t, bc_b, agi, ago, h_next):
                """h_fm: [128, SH] feature-major input (dtype matches wc_t).
                h_next: [128, SH] bf16 feature-major output tile."""
                # transform (feature-major)
                g_fm = actp.tile([128, SH], bf16, tag="act", name="g_fm")
                for s0, sw in slices:
                    ps = mmp.tile([128, 512], f32, space="PSUM", tag="mm")
                    nc.tensor.matmul(ps[:, :sw], lhsT=wc_t[:],
                                     rhs=h_fm[:, s0:s0 + sw],
                                     start=True, stop=True)
                    nc.scalar.activation(g_fm[:, s0:s0 + sw], ps[:, :sw], COPY)
                # node-major bf16 dinv-scaled shard, SBUF-resident; store + AllGather
                sh_sb = shp_.tile([128, NT * 128], bf16, tag="sh", name="sh_sb")
                for t in range(NT):
                    pt = trp.tile([128, 128], bf16, space="PSUM", tag="tr")
                    nc.tensor.transpose(out=pt[:],
                                        in_=g_fm[:, t * 128:(t + 1) * 128],
                                        identity=ident_bf[:])
                    nc.vector.tensor_scalar_mul(
                        sh_sb[:, t * 128:(t + 1) * 128], pt[:],
                        dinv_sb[:, t:t + 1])
                    nc.sync.dma_start(
                        out=agi[t * 128:(t + 1) * 128, :],
                        in_=sh_sb[:, t * 128:(t + 1) * 128])
                nc.gpsimd.collective_compute(
                    "AllGather", mybir.AluOpType.bypass,
                    replica_groups=[list(range(NCORES))],
                    ins=[agi.opt()], outs=[ago.opt()],
                )
                # aggregation: per 128-dst tile
                for t in range(NT):
                    nch = int(CH[t])
                    c_lo = int(chunk_off[t])
                    pa = aggp.tile([128, 128], f32, space="PSUM", tag="agg")
                    for j in range(nch):
                        cid = c_lo + j
                        g_st = gatp.tile([128, 128], bf16, tag="g", name="g_st")
                        nc.gpsimd.indirect_dma_start(
                            out=g_st[:], out_offset=None, in_=ago[:],
                            in_offset=bass.IndirectOffsetOnAxis(
                                ap=idx_sb[:, cid:cid + 1], axis=0))
                        oh = ohp.tile([128, 128], bf16, tag="oh", name="oh")
                        nc.vector.tensor_tensor(
                            out=oh[:],
                            in0=doff_sb[:, cid:cid + 1].to_broadcast([128, 128]),
                            in1=iota_sb[:],
                            op=EQ)
                        nc.tensor.matmul(pa[:], lhsT=oh[:], rhs=g_st[:],
                                         start=(j == 0), stop=(j == nch - 1))
                    # evacuate: relu(dinv_dst*(agg + self) + bias)
                    gl = smp.tile([128, 128], f32, tag="sm", name="gl")
                    nc.scalar.activation(gl[:], sh_sb[:, t * 128:(t + 1) * 128],
                                         COPY)
                    ev = smp.tile([128, 128], f32, tag="sm", name="ev")
                    nc.vector.tensor_add(ev[:], pa[:], gl[:])
                    nc.vector.tensor_scalar_mul(ev[:], ev[:],
                                                dinv_sb[:, t:t + 1])
                    nc.vector.tensor_add(ev[:], ev[:], bc_b[:])
                    nc.vector.tensor_relu(ev[:], ev[:])
                    # inline transpose -> feature-major bf16 for next stage
                    pt = trp.tile([128, 128], f32, space="PSUM", tag="tr")
                    nc.tensor.transpose(out=pt[:], in_=ev[:], identity=ident[:])
                    nc.scalar.activation(h_next[:, t * 128:(t + 1) * 128],
                                         pt[:], COPY)

            hE = hbp.tile([128, SH], bf16, tag="hbf", name="hE")
            conv(hD, W["wc1T"], W["bc1b"], ag_in, ag_out, hE)
            hF = hbp.tile([128, SH], bf16, tag="hbf", name="hF")
            conv(hE, W["wc2T"], W["bc2b"], ag_in2, ag_out2, hF)

            # final head: out = h6 @ W5.T + b5  -> [SH, 60]
            for s0, sw in slices:
                ps = mmp.tile([128, 512], f32, space="PSUM", tag="mm")
                nc.tensor.matmul(ps[:60, :sw], lhsT=W["w5T"][:],
                                 rhs=hF[:, s0:s0 + sw], start=True, stop=True)
                of = xsp.tile([60, 512], f32, tag="of", name="of")
                nc.vector.tensor_scalar_add(of[:, :sw], ps[:60, :sw],
                                            W["b5c"][:])
                for q in range(0, sw, 128):
                    qw = min(128, sw - q)
                    pt = trp.tile([128, 128], f32, space="PSUM", tag="tr")
                    nc.tensor.transpose(out=pt[:qw, :60], in_=of[:60, q:q + qw],
                                        identity=ident[:60, :60])
                    on = smp.tile([128, 60], f32, tag="on", name="on")
                    nc.vector.tensor_copy(on[:qw, :], pt[:qw, :60])
                    nc.sync.dma_start(out=out[s0 + q:s0 + q + qw, :],
                                      in_=on[:qw, :])
    nc.compile()
    return nc


def kernel(x, edge_index, W1, b1, W2, b2, W3, b3, W4, b4,
           Wc1, bc1, Wc2, bc2, W5, b5):
    import ml_dtypes
    from concourse.bass_utils import run_bass_kernel_spmd

    x = np.asarray(x, dtype=np.float32)
    key = "k"
    if key not in _cache:
        dinv, TOTCH, CH, chunk_off, idxs, dstoff = _prep(np.asarray(edge_index))
        nc = _build(TOTCH, CH, chunk_off)
        _cache[key] = (dinv, TOTCH, idxs, dstoff, nc)
    dinv, TOTCH, idxs, dstoff, nc = _cache[key]

    xp = np.zeros((N_PAD, 5), dtype=np.float32)
    xp[:N_NODES] = x
    bf = ml_dtypes.bfloat16
    iota = np.tile(np.arange(128, dtype=np.float32).astype(bf)[None, :],
                   (128, 1))
    in_maps = []
    for c in range(NCORES):
        sl = slice(c * SH, (c + 1) * SH)
        m = {
            "xT": np.ascontiguousarray(xp[sl].T).astype(bf),
            "idxs": idxs[c],
            "dstoff": dstoff[c],
            "dinv_cols": np.ascontiguousarray(dinv[sl].reshape(NT, 128).T),
            "iota_t": iota,
            "w1T": np.ascontiguousarray(np.asarray(W1, np.float32).T).astype(bf),
            "w2T": np.ascontiguousarray(np.asarray(W2, np.float32).T).astype(bf),
            "w3T": np.ascontiguousarray(np.asarray(W3, np.float32).T).astype(bf),
            "w4T": np.ascontiguousarray(np.asarray(W4, np.float32).T).astype(bf),
            "wc1T": np.ascontiguousarray(np.asarray(Wc1, np.float32).T).astype(bf),
            "wc2T": np.ascontiguousarray(np.asarray(Wc2, np.float32).T).astype(bf),
            "w5T": np.ascontiguousarray(np.asarray(W5, np.float32).T).astype(bf),
            "b1c": np.asarray(b1, np.float32)[:, None],
            "b2c": np.asarray(b2, np.float32)[:, None],
            "b3c": np.asarray(b3, np.float32)[:, None],
            "b4c": np.asarray(b4, np.float32)[:, None],
            "b5c": np.asarray(b5, np.float32)[:, None],
            "bc1b": np.tile(np.asarray(bc1, np.float32)[None, :], (128, 1)),
            "bc2b": np.tile(np.asarray(bc2, np.float32)[None, :], (128, 1)),
        }
        in_maps.append(m)
    res = run_bass_kernel_spmd(nc, in_maps, list(range(NCORES)), trace=TRACE)
    globals()["LAST"] = res
    outs = [res.results[c]["out"] for c in range(NCORES)]
    return np.concatenate(outs, axis=0)[:N_NODES]


# revision 24
# speedup vs baseline: 1.0039x; 1.0039x over previous
"""GNN (MLP + 2x GCNConv + head) on 8 Trainium2 NeuronCores.

Sharding: nodes split 8 ways (12544 per core, padded from 100000 to 100352).
Per conv: transform on PE (feature-major), scale cols by dinv, PE-transpose
to node-major, convert to bf16, AllGather the bf16 table, then per 128-dst
tile: indirect-DMA gather of 128 source rows per chunk (bf16, 256B rows),
one-hot built ON-CHIP by DVE (is_equal vs iota) from a packed dst-offset
stream, scatter-add via bf16 matmul into a [128dst,128feat] PSUM tile,
evacuation adds self-loop from the SBUF-resident local shard + bias + relu,
then inline PE-transpose feeds the next stage (no DRAM round trip).
All edge bookkeeping (dst-sorted chunked index/offset streams) precomputed
on host at first call and cached.
"""
import numpy as np

N_NODES = 100000
N_PAD = 100352          # 8 * 12544
SH = 12544              # nodes per core (98 tiles of 128)
NT = 98                 # 128-node tiles per core
CHUNK = 128             # edges per matmul chunk
HID = 128
NCORES = 8

_cache = {}
TRACE = False      # set True (e.g. from test.py) to capture an NTFF profile
LAST = None        # BassKernelResults of the most recent run


def _prep(edge_index):
    src = np.asarray(edge_index[0], dtype=np.int64)
    dst = np.asarray(edge_index[1], dtype=np.int64)
    deg = np.bincount(dst, minlength=N_PAD).astype(np.float64) + 1.0
    dinv = (1.0 / np.sqrt(deg)).astype(np.float32)  # pad nodes -> 1.0

    core_of = dst // SH
    ch_t = np.zeros((NCORES, NT), dtype=np.int64)
    edata = []
    for c in range(NCORES):
        m = core_of == c
        s = src[m]
        dl = dst[m] - c * SH
        o = np.argsort(dl, kind="stable")
        s, dl = s[o], dl[o]
        t = dl // 128
        cnt = np.bincount(t, minlength=NT)
        ch_t[c] = (cnt + CHUNK - 1) // CHUNK
        edata.append((s, dl, cnt))
    CH = np.maximum(ch_t.max(axis=0), 1)        # chunks per tile (shared)
    TOTCH = int(CH.sum())
    chunk_off = np.concatenate([[0], np.cumsum(CH)])

    import ml_dtypes
    idxs = np.zeros((NCORES, 128, TOTCH), dtype=np.int32)
    dstoff = np.full((NCORES, 128, TOTCH), 128.0, dtype=ml_dtypes.bfloat16)
    for c in range(NCORES):
        s, dl, cnt = edata[c]
        tstart = np.concatenate([[0], np.cumsum(cnt)])
        pos_in_t = np.arange(len(dl)) - tstart[dl // 128]
        ch_local = pos_in_t // CHUNK
        lane = pos_in_t % CHUNK
        gch = chunk_off[dl // 128] + ch_local
        idxs[c, lane, gch] = s.astype(np.int32)
        dstoff[c, lane, gch] = (dl % 128).astype(np.float32)
    return dinv, TOTCH, CH, chunk_off, idxs, dstoff


def _build(TOTCH, CH, chunk_off):
    import concourse.bacc as bacc
    import concourse.bass as bass
    import concourse.mybir as mybir
    import concourse.tile as tile
    from concourse.masks import make_identity

    f32 = mybir.dt.float32
    bf16 = mybir.dt.bfloat16
    i32 = mybir.dt.int32
    RELU = mybir.ActivationFunctionType.Relu
    COPY = mybir.ActivationFunctionType.Copy
    EQ = mybir.AluOpType.is_equal

    nc = bacc.Bacc("TRN2", target_bir_lowering=False, debug=False,
                   enable_asserts=False, num_devices=NCORES)

    xT = nc.dram_tensor("xT", [5, SH], bf16, kind="ExternalInput")
    idxs = nc.dram_tensor("idxs", [128, TOTCH], i32, kind="ExternalInput")
    dstoff = nc.dram_tensor("dstoff", [128, TOTCH], bf16, kind="ExternalInput")
    dinv_cols = nc.dram_tensor("dinv_cols", [128, NT], f32, kind="ExternalInput")
    iota_t = nc.dram_tensor("iota_t", [128, 128], bf16, kind="ExternalInput")
    wts = {}
    for nm, shp, dt in [
            ("w1T", [5, 64], bf16), ("w2T", [64, 128], bf16),
            ("w3T", [128, 128], bf16), ("w4T", [128, 128], bf16),
            ("wc1T", [128, 128], bf16), ("wc2T", [128, 128], bf16),
            ("w5T", [128, 60], bf16),
            ("b1c", [64, 1], f32), ("b2c", [128, 1], f32),
            ("b3c", [128, 1], f32), ("b4c", [128, 1], f32),
            ("b5c", [60, 1], f32),
            ("bc1b", [128, 128], f32), ("bc2b", [128, 128], f32)]:
        wts[nm] = nc.dram_tensor(nm, shp, dt, kind="ExternalInput")
    out = nc.dram_tensor("out", [SH, 60], f32, kind="ExternalOutput")

    with tile.TileContext(nc) as tc:
        with tc.tile_pool(name="w", bufs=1) as wp, \
             tc.tile_pool(name="act", bufs=2) as actp, \
             tc.tile_pool(name="hb", bufs=2) as hbp, \
             tc.tile_pool(name="sh", bufs=2) as shp_, \
             tc.tile_pool(name="xs", bufs=2) as xsp, \
             tc.tile_pool(name="sm", bufs=4) as smp, \
             tc.tile_pool(name="oh", bufs=4) as ohp, \
             tc.tile_pool(name="gat", bufs=32) as gatp, \
             tc.tile_pool(name="mm", bufs=2, space="PSUM") as mmp, \
             tc.tile_pool(name="tr", bufs=2, space="PSUM") as trp, \
             tc.tile_pool(name="agg", bufs=4, space="PSUM") as aggp, \
             tc.tile_pool(name="dram", bufs=1, space="DRAM") as dramp:

            W = {}
            for nm in wts:
                W[nm] = wp.tile(list(wts[nm].shape), wts[nm].dtype,
                                tag=nm, name=nm + "_sb")
                nc.sync.dma_start(out=W[nm][:], in_=wts[nm][:])
            dinv_sb = wp.tile([128, NT], f32, tag="dinv", name="dinv_sb")
            nc.sync.dma_start(out=dinv_sb[:], in_=dinv_cols[:])
            iota_sb = wp.tile([128, 128], bf16, tag="iota", name="iota_sb")
            nc.sync.dma_start(out=iota_sb[:], in_=iota_t[:])
            ident = wp.tile([128, 128], f32, tag="ident", name="ident")
            make_identity(nc, ident[:])
            ident_bf = wp.tile([128, 128], bf16, tag="identb", name="ident_bf")
            nc.scalar.activation(ident_bf[:], ident[:], COPY)
            idx_sb = wp.tile([128, TOTCH], i32, tag="idx", name="idx_sb")
            nc.sync.dma_start(out=idx_sb[:], in_=idxs[:])
            doff_sb = wp.tile([128, TOTCH], bf16, tag="doff", name="doff_sb")
            nc.sync.dma_start(out=doff_sb[:], in_=dstoff[:])

            ag_in = dramp.tile([SH, HID], bf16, name="ag_in")
            ag_out = dramp.tile([N_PAD, HID], bf16, name="ag_out",
                                addr_space="Shared")
            ag_in2 = dramp.tile([SH, HID], bf16, name="ag_in2")
            ag_out2 = dramp.tile([N_PAD, HID], bf16, name="ag_out2",
                                 addr_space="Shared")

            slices = [(s, min(512, SH - s)) for s in range(0, SH, 512)]

            def mlp_layer(dst_t, w_t, b_t, src_t, kin, kout, resid=None):
                for s0, sw in slices:
                    ps = mmp.tile([128, 512], f32, space="PSUM", tag="mm")
                    nc.tensor.matmul(ps[:kout, :sw], lhsT=w_t[:],
                                     rhs=src_t[:kin, s0:s0 + sw],
                                     start=True, stop=True)
                    nc.scalar.activation(dst_t[:kout, s0:s0 + sw],
                                         ps[:kout, :sw], RELU, bias=b_t[:])
                    if resid is not None:
                        nc.vector.tensor_add(dst_t[:kout, s0:s0 + sw],
                                             dst_t[:kout, s0:s0 + sw],
                                             resid[:kout, s0:s0 + sw])

            # ---- MLP (feature-major, fp32) ----
            hA = actp.tile([128, SH], bf16, tag="act", name="hA")
            for s0, sw in slices:
                xt = xsp.tile([5, 512], bf16, tag="xs", name="xt")
                nc.sync.dma_start(out=xt[:, :sw], in_=xT[:, s0:s0 + sw])
                ps = mmp.tile([128, 512], f32, space="PSUM", tag="mm")
                nc.tensor.matmul(ps[:64, :sw], lhsT=W["w1T"][:], rhs=xt[:5, :sw],
                                 start=True, stop=True)
                nc.scalar.activation(hA[:64, s0:s0 + sw], ps[:64, :sw], RELU,
                                     bias=W["b1c"][:])
            hB = actp.tile([128, SH], bf16, tag="act", name="hB")
            mlp_layer(hB, W["w2T"], W["b2c"], hA, 64, 128)
            hC = actp.tile([128, SH], bf16, tag="act", name="hC")
            mlp_layer(hC, W["w3T"], W["b3c"], hB, 128, 128, resid=hB)
            hD = actp.tile([128, SH], bf16, tag="act", name="hD")
            mlp_layer(hD, W["w4T"], W["b4c"], hC, 128, 128, resid=hC)

            def conv(h_fm, wc_t, bc_b, agi, ago, h_next):
                """h_fm: [128, SH] feature-major input (dtype matches wc_t).
                h_next: [128, SH] bf16 feature-major output tile."""
                # transform (feature-major)
                g_fm = actp.tile([128, SH], bf16, tag="act", name="g_fm")
                for s0, sw in slices:
                    ps = mmp.tile([128, 512], f32, space="PSUM", tag="mm")
                    nc.tensor.matmul(ps[:, :sw], lhsT=wc_t[:],
                                     rhs=h_fm[:, s0:s0 + sw],
                                     start=True, stop=True)
                    nc.scalar.activation(g_fm[:, s0:s0 + sw], ps[:, :sw], COPY)
                # node-major bf16 dinv-scaled shard, SBUF-resident; store + AllGather
                sh_sb = shp_.tile([128, NT * 128], bf16, tag="sh", name="sh_sb")
                for t in range(NT):
                    pt = trp.tile([128, 128], bf16, space="PSUM", tag="tr")
                    nc.tensor.transpose(out=pt[:],
                                        in_=g_fm[:, t * 128:(t + 1) * 128],
                                        identity=ident_bf[:])
                    nc.vector.tensor_scalar_mul(
                        sh_sb[:, t * 128:(t + 1) * 128], pt[:],
                        dinv_sb[:, t:t + 1])
                    nc.sync.dma_start(
                        out=agi[t * 128:(t + 1) * 128, :],
                        in_=sh_sb[:, t * 128:(t + 1) * 128])
                nc.gpsimd.collective_compute(
                    "AllGather", mybir.AluOpType.bypass,
                    replica_groups=[list(range(NCORES))],
                    ins=[agi.opt()], outs=[ago.opt()],
                )
                # aggregation: per 128-dst tile
                for t in range(NT):
                    nch = int(CH[t])
                    c_lo = int(chunk_off[t])
                    pa = aggp.tile([128, 128], f32, space="PSUM", tag="agg")
                    for j in range(nch):
                        cid = c_lo + j
                        g_st = gatp.tile([128, 128], bf16, tag="g", name="g_st")
                        nc.gpsimd.indirect_dma_start(
                            out=g_st[:], out_offset=None, in_=ago[:],
                            in_offset=bass.IndirectOffsetOnAxis(
                                ap=idx_sb[:, cid:cid + 1], axis=0))
                        oh = ohp.tile([128, 128], bf16, tag="oh", name="oh")
                        nc.vector.tensor_tensor(
                            out=oh[:],
                            in0=doff_sb[:, cid:cid + 1].to_broadcast([128, 128]),
                            in1=iota_sb[:],
                            op=EQ)
                        nc.tensor.matmul(pa[:], lhsT=oh[:], rhs=g_st[:],
                                         start=(j == 0), stop=(j == nch - 1))
                    # evacuate: relu(dinv_dst*(agg + self) + bias)
                    gl = smp.tile([128, 128], f32, tag="sm", name="gl")
                    nc.scalar.activation(gl[:], sh_sb[:, t * 128:(t + 1) * 128],
                                         COPY)
                    ev = smp.tile([128, 128], f32, tag="sm", name="ev")
                    nc.vector.tensor_add(ev[:], pa[:], gl[:])
                    nc.vector.tensor_scalar_mul(ev[:], ev[:],
                                                dinv_sb[:, t:t + 1])
                    nc.vector.tensor_add(ev[:], ev[:], bc_b[:])
                    nc.vector.tensor_relu(ev[:], ev[:])
                    # inline transpose -> feature-major bf16 for next stage
                    pt = trp.tile([128, 128], f32, space="PSUM", tag="tr")
                    nc.tensor.transpose(out=pt[:], in_=ev[:], identity=ident[:])
                    nc.scalar.activation(h_next[:, t * 128:(t + 1) * 128],
                                         pt[:], COPY)

            hE = hbp.tile([128, SH], bf16, tag="hbf", name="hE")
            conv(hD, W["wc1T"], W["bc1b"], ag_in, ag_out, hE)
            hF = hbp.tile([128, SH], bf16, tag="hbf", name="hF")
            conv(hE, W["wc2T"], W["bc2b"], ag_in2, ag_out2, hF)

            # final head: out = h6 @ W5.T + b5  -> [SH, 60]
            for s0, sw in slices:
                ps = mmp.tile([128, 512], f32, space="PSUM", tag="mm")
                nc.tensor.matmul(ps[:60, :sw], lhsT=W["w5T"][:],
                                 rhs=hF[:, s0:s0 + sw], start=True, stop=True)
                of = xsp.tile([60, 512], f32, tag="of", name="of")
                nc.vector.tensor_scalar_add(of[:, :sw], ps[:60, :sw],
                                            W["b5c"][:])
                for q in range(0, sw, 128):
                    qw = min(128, sw - q)
                    pt = trp.tile([128, 128], f32, space="PSUM", tag="tr")
                    nc.tensor.transpose(out=pt[:qw, :60], in_=of[:60, q:q + qw],
                                        identity=ident[:60, :60])
                    on = smp.tile([128, 60], f32, tag="on", name="on")
                    nc.vector.tensor_copy(on[:qw, :], pt[:qw, :60])
                    nc.sync.dma_start(out=out[s0 + q:s0 + q + qw, :],
                                      in_=on[:qw, :])
    nc.compile()
    return nc


def kernel(x, edge_index, W1, b1, W2, b2, W3, b3, W4, b4,
           Wc1, bc1, Wc2, bc2, W5, b5):
    import ml_dtypes
    from concourse.bass_utils import run_bass_kernel_spmd

    x = np.asarray(x, dtype=np.float32)
    key = "k"
    if key not in _cache:
        dinv, TOTCH, CH, chunk_off, idxs, dstoff = _prep(np.asarray(edge_index))
        nc = _build(TOTCH, CH, chunk_off)
        _cache[key] = (dinv, TOTCH, idxs, dstoff, nc)
    dinv, TOTCH, idxs, dstoff, nc = _cache[key]

    xp = np.zeros((N_PAD, 5), dtype=np.float32)
    xp[:N_NODES] = x
    bf = ml_dtypes.bfloat16
    iota = np.tile(np.arange(128, dtype=np.float32).astype(bf)[None, :],
                   (128, 1))
    in_maps = []
    for c in range(NCORES):
        sl = slice(c * SH, (c + 1) * SH)
        m = {
            "xT": np.ascontiguousarray(xp[sl].T).astype(bf),
            "idxs": idxs[c],
            "dstoff": dstoff[c],
            "dinv_cols": np.ascontiguousarray(dinv[sl].reshape(NT, 128).T),
            "iota_t": iota,
            "w1T": np.ascontiguousarray(np.asarray(W1, np.float32).T).astype(bf),
            "w2T": np.ascontiguousarray(np.asarray(W2, np.float32).T).astype(bf),
            "w3T": np.ascontiguousarray(np.asarray(W3, np.float32).T).astype(bf),
            "w4T": np.ascontiguousarray(np.asarray(W4, np.float32).T).astype(bf),
            "wc1T": np.ascontiguousarray(np.asarray(Wc1, np.float32).T).astype(bf),
            "wc2T": np.ascontiguousarray(np.asarray(Wc2, np.float32).T).astype(bf),
            "w5T": np.ascontiguousarray(np.asarray(W5, np.float32).T).astype(bf),
            "b1c": np.asarray(b1, np.float32)[:, None],
            "b2c": np.asarray(b2, np.float32)[:, None],
            "b3c": np.asarray(b3, np.float32)[:, None],
            "b4c": np.asarray(b4, np.float32)[:, None],
            "b5c": np.asarray(b5, np.float32)[:, None],
            "bc1b": np.tile(np.asarray(bc1, np.float32)[None, :], (128, 1)),
            "bc2b": np.tile(np.asarray(bc2, np.float32)[None, :], (128, 1)),
        }
        in_maps.append(m)
    res = run_bass_kernel_spmd(nc, in_maps, list(range(NCORES)), trace=TRACE)
    globals()["LAST"] = res
    outs = [res.results[c]["out"] for c in range(NCORES)]
    return np.concatenate(outs, axis=0)[:N_NODES]
